# revision 2
# baseline (speedup 1.0000x reference)
"""NT-Xent loss on 8 Trainium2 cores (v5: dual-engine fused exp).

Structure (per core (v,s), cyclic 3-block symmetry as v4): slab s's
1024 rows against column blocks [s | s+2 | s+1] (3072 cols).  The v4
bottleneck was ScalarE doing ALL 3.1M exps (25us) while DVE spent 23us
on reductions.  v5 splits the exp work across both engines, each with a
FUSED free-dim accumulator, and moves the +1-block column sums to the
PE:

- psB half [+2b 512 | +1 1024]: ScalarE ACT Exp -> fp8, accum_out =
  rowsum (pre-quantization fp32, verified on HW).
- psA half [own 1024 | +2a 512]: custom DVE op EXP8_SUM_ANT
  (body = sq(sq(sq((x*c0 + c1)*x + c2))) ~= lam*e^(x*ASC), fitted;
  7 ALU stages + accum) -> fp8 scratch, accum_out = rowsum.
- +1-block colsums: paired DoubleRow ones-matmuls (ones [128,2,128]
  weight, rhs = two i-tiles' fp8 exp) accumulated across the 4 i-tile
  pairs into 2 persistent PSUM banks (128-replicated rows).

Host: rowsum = (accD/lam + accA) - diag + colsum[(s-1)%4]; diag is
emulated exactly (EXP8 is bit-exact vs np.float32 emulation, verified
on HW).  Shift sigma=0 so exp values are fp8-friendly (e^s, s in
[-3.3, 3.3] off-diagonal); lse = ln(rowsum + e^pos).
"""

import numpy as np
import ml_dtypes

N = 4096
D = 256
TEMP = 0.1
NCORES = 8
RPC = 2 * N // NCORES          # 1024 rows per core
IT = RPC // 128                # 8 i-tiles of 128 rows
W = 3 * RPC                    # 3072 columns per core
HALFW = W // 2                 # 1536 cols per PSUM buffer
NCH = HALFW // 512             # 3 column chunks per half
SC = 16.0                      # fp8 prescale (power of 2, exact)
ASCALE = (1.0 / TEMP) / (SC * SC)   # 10/256, exact in fp32

# EXP8 fit: (C2S + C1S*s + C0S*s^2)^8 ~= LAM * e^s  (s = G*ASCALE),
# weighted by N(0,0.625)*e^s over s in [-3.45, 3.45]
C0S, C1S, C2S = 0.00812527624, 0.125296963, 0.999881204
LAM = 1.00007132
C0G = float(np.float32(C0S * ASCALE * ASCALE))
C1G = float(np.float32(C1S * ASCALE))
C2G = float(np.float32(C2S))

_CACHE = {}


def _register_exp8():
    """Register the EXP8_SUM_ANT custom DVE op (runtime equivalent of the
    documented OPS.append flow; sha computed from the lowered uops)."""
    from operator import add
    from concourse.dve_spec import Spec, Src0, C0, C1, C2, Zero, sq, lower
    from concourse.dve_uop import DveOpSpec
    import concourse.dve_ops as dom

    name = "EXP8_SUM_ANT"
    for op in dom.OPS:
        if op.name == name:
            return op

    body = sq(sq(sq((Src0 * C0 + C1) * Src0 + C2)))

    def ref(in0, in1, s0, s1, imm2):
        x = in0.astype(np.float32)
        t = ((x * np.float32(s0) + np.float32(s1)) * x + np.float32(imm2)).astype(
            np.float32
        )
        t = (t * t).astype(np.float32)
        t = (t * t).astype(np.float32)
        t = (t * t).astype(np.float32)
        return t, t.reshape(t.shape[0], -1).sum(axis=-1, keepdims=True).astype(
            np.float32
        )

    spec = Spec(body=body, accum=add, accum_init=Zero, reference=ref)
    row = dom._CUSTOM_DVE_ROW_BASE + len(dom.OPS)
    dom._SUB_OPCODE_FOR_NAME[name] = row
    shas = {}
    for ver in ("v3", "v4"):
        shas[ver] = DveOpSpec(
            name=name, opcode=row, uops=lower(spec, ver=ver), rd1_en=False
        ).sha(ver)
    op = dom.DveOp(name, spec, subdim=False, uops_sha=shas)
    dom.OPS.append(op)
    dom.CUSTOM_DVE_SPECS[name] = spec
    return op


def _exp8_host(x):
    """Bit-exact host emulation of the device EXP8 body (fp32 stages)."""
    x = np.asarray(x, dtype=np.float32)
    t = ((x * np.float32(C0G) + np.float32(C1G)) * x + np.float32(C2G)).astype(
        np.float32
    )
    t = (t * t).astype(np.float32)
    t = (t * t).astype(np.float32)
    t = (t * t).astype(np.float32)
    return t


def _build_program():
    if "nc" in _CACHE:
        return _CACHE["nc"]

    import concourse.tile as tile
    from concourse import bacc, mybir

    EXP8 = _register_exp8()

    F8 = mybir.dt.float8e4
    F32 = mybir.dt.float32

    nc = bacc.Bacc(
        "TRN2", target_bir_lowering=False, debug=False, num_devices=NCORES
    )

    # anT[h][c][p][k][col] = cols[h*1536 + c*512 + col, k*128 + p]
    # column order per core: [own 1024 | +2 1024 | +1 1024]
    anT_d = nc.dram_tensor("anT", [2, NCH, 128, 2, 512], F8, kind="ExternalInput")
    # qnT[p][k][r] = q8slab[r, k*128 + p]
    qnT_d = nc.dram_tensor("qnT", [128, 2, RPC], F8, kind="ExternalInput")
    acc_d = nc.dram_tensor("acc", [128, IT, 2], F32, kind="ExternalOutput")
    cs_d = nc.dram_tensor("cs", [1, RPC], F32, kind="ExternalOutput")

    with tile.TileContext(nc) as tc:
        with (
            tc.tile_pool(name="weights", bufs=1) as wpool,
            tc.tile_pool(name="scratch", bufs=2) as spool,
            tc.tile_pool(name="psum", bufs=2, space="PSUM") as ppool,
            tc.tile_pool(name="pcs", bufs=1, space="PSUM") as cpool,
        ):
            qnT = wpool.tile([128, 2, RPC], F8)
            an = [
                [wpool.tile([128, 2, 512], F8, name=f"an{h}_{c}") for c in range(NCH)]
                for h in range(2)
            ]
            # early queues feed tile 0's gates first: qnT's first 128 cols
            # and hB (psB half is consumed first)
            nc.sync.dma_start(out=qnT[:, :, 0:128], in_=qnT_d[:, :, 0:128])
            nc.scalar.dma_start(out=an[1][0][:], in_=anT_d[1, 0])
            nc.gpsimd.dma_start(out=an[1][1][:], in_=anT_d[1, 1])
            nc.sync.dma_start(out=qnT[:, :, 128:RPC], in_=qnT_d[:, :, 128:RPC])
            nc.scalar.dma_start(out=an[1][2][:], in_=anT_d[1, 2])
            nc.gpsimd.dma_start(out=an[0][0][:], in_=anT_d[0, 0])
            nc.sync.dma_start(out=an[0][1][:], in_=anT_d[0, 1])
            nc.scalar.dma_start(out=an[0][2][:], in_=anT_d[0, 2])

            acc = wpool.tile([128, IT, 2], F32)
            csb = wpool.tile([128, RPC], F32)
            scrD = wpool.tile([128, HALFW], F8)
            w1 = wpool.tile([128, 2, 128], F8)
            nc.vector.memset(w1[:], 1.0)

            # persistent colsum accumulator: 2 banks, 128-replicated rows
            cs = cpool.tile([128, RPC], F32)

            # two garbage DoubleRow MMs take the PE out of its cold state;
            # they write into cs, which the first real ones-MM (start=True)
            # later resets
            with tc.high_priority():
                for _ in range(2):
                    nc.tensor.matmul(
                        cs[:, 0:128],
                        w1[:],
                        w1[:],
                        start=True,
                        stop=True,
                        perf_mode=mybir.MatmulPerfMode.DoubleRow,
                        skip_group_check=True,
                    )

            e1 = None
            for t in range(IT):
                lhsT = qnT[:, :, t * 128:(t + 1) * 128]

                # ---- psB half [+2b 512 | +1 1024] -> ScalarE exp
                psB = ppool.tile([128, HALFW], F32, tag="ps")
                for c in range(NCH):
                    nc.tensor.matmul(
                        psB[:, c * 512:(c + 1) * 512],
                        lhsT,
                        an[1][c][:],
                        start=True,
                        stop=True,
                        perf_mode=mybir.MatmulPerfMode.DoubleRow,
                    )
                if t % 2 == 0:
                    e1 = spool.tile([128, 2, HALFW], F8, tag="e1")
                assert e1 is not None
                nc.scalar.activation(
                    e1[:, t % 2, :],
                    psB[:],
                    mybir.ActivationFunctionType.Exp,
                    bias=0.0,
                    scale=float(ASCALE),
                    accum_out=acc[:, t, 0:1],
                )

                # ---- psA half [own 1024 | +2a 512] -> DVE EXP8
                psA = ppool.tile([128, HALFW], F32, tag="ps")
                for c in range(NCH):
                    nc.tensor.matmul(
                        psA[:, c * 512:(c + 1) * 512],
                        lhsT,
                        an[0][c][:],
                        start=True,
                        stop=True,
                        perf_mode=mybir.MatmulPerfMode.DoubleRow,
                    )
                nc.vector._custom_dve(
                    EXP8,
                    out=scrD[:],
                    in0=psA[:],
                    s0=C0G,
                    s1=C1G,
                    imm2=C2G,
                    accum_out=acc[:, t, 1:2],
                )

                # ---- paired ones-MMs: colsums of the +1 block (cols
                # 512:1536 of each e1 slot), accumulated across pairs
                if t % 2 == 1:
                    for half in range(2):
                        nc.tensor.matmul(
                            cs[:, half * 512:(half + 1) * 512],
                            w1[:],
                            e1[:, :, 512 + half * 512:1024 + half * 512],
                            start=(t == 1),
                            stop=(t == IT - 1),
                            perf_mode=mybir.MatmulPerfMode.DoubleRow,
                            skip_group_check=True,
                        )

            # evacuate colsums (PSUM can't DMA): split across both engines
            nc.scalar.activation(
                csb[:, 0:512],
                cs[:, 0:512],
                mybir.ActivationFunctionType.Copy,
            )
            nc.vector.tensor_copy(csb[:, 512:RPC], cs[:, 512:RPC])

            nc.sync.dma_start(out=acc_d[:], in_=acc[:])
            nc.gpsimd.dma_start(out=cs_d[:], in_=csb[0:1, :])

    nc.compile()
    _CACHE["nc"] = nc
    return nc


def _prep_inputs(z_i, z_j):
    f8 = ml_dtypes.float8_e4m3
    zin = z_i / np.sqrt(np.sum(z_i * z_i, axis=1, keepdims=True))
    zjn = z_j / np.sqrt(np.sum(z_j * z_j, axis=1, keepdims=True))
    posn = np.sum(zin * zjn, axis=1, dtype=np.float64) / TEMP      # [4096]

    q8 = [(SC * zjn).astype(f8), (SC * zin).astype(f8)]
    # exact squared norms of the quantized rows: the device Gram diagonal
    dsq = [np.sum(b.astype(np.float64) ** 2, axis=1) for b in q8]

    in_maps = []
    for c in range(NCORES):
        v, s = divmod(c, NCORES // 2)
        b = q8[v]
        brot = np.roll(b, -s * RPC, axis=0)
        # column order: [own | +2 | +1]; +1 sits in psB at local cols
        # 512:1536 so the ones-MMs read e1[:, :, 512:1536]
        cols = np.concatenate(
            [brot[0:RPC], brot[2 * RPC:3 * RPC], brot[RPC:2 * RPC]], axis=0
        )                                               # [3072, 256]
        anT = np.ascontiguousarray(
            cols.T.reshape(2, 128, 2, NCH, 512).transpose(2, 3, 1, 0, 4)
        )
        slab = b[s * RPC:(s + 1) * RPC]
        qnT = np.ascontiguousarray(slab.T.reshape(2, 128, RPC).transpose(1, 0, 2))
        in_maps.append({"anT": anT, "qnT": qnT})
    return in_maps, posn, dsq


def kernel(z_i, z_j):
    z_i = np.asarray(z_i, dtype=np.float32)
    z_j = np.asarray(z_j, dtype=np.float32)

    from concourse.bass_utils import run_bass_kernel_spmd

    nc = _build_program()
    in_maps, posn, dsq = _prep_inputs(z_i, z_j)

    res = run_bass_kernel_spmd(nc, in_maps, list(range(NCORES)))
    _CACHE["last_results"] = res

    nv = NCORES // 2
    rowsum = np.empty(2 * N, dtype=np.float64)
    colsum = np.empty((2, nv, RPC), dtype=np.float64)
    for c in range(NCORES):
        v, s = divmod(c, nv)
        a = res.results[c]["acc"].astype(np.float64)   # [128, IT, 2]
        # acc[:, t, 0] = ACT rowsum (psB), acc[:, t, 1] = EXP8 rowsum (psA)
        rs = a[:, :, 0] + a[:, :, 1] / LAM             # [128, IT]
        rowsum[c * RPC:(c + 1) * RPC] = rs.T.reshape(-1)
        colsum[v, s] = res.results[c]["cs"].astype(np.float64)[0]
    for v in range(2):
        for s in range(nv):
            # slab s's missing (s, s+3) block rowsums = colsums of the
            # +1 block computed by core (v, s-1)
            g0 = v * N + s * RPC
            rowsum[g0:g0 + RPC] += colsum[v, (s - 1) % nv]

    # exact diagonal removal: the diagonal sits in the own block (psA ->
    # EXP8); emulate the device computation bit-exactly
    dsq_g = np.concatenate(dsq).astype(np.float32)     # [8192] |q8 row|^2
    rowsum -= _exp8_host(dsq_g).astype(np.float64) / LAM

    posn_g = np.concatenate([posn, posn])
    epos_g = np.exp(posn_g)

    lse = np.log(rowsum + epos_g)
    loss = np.mean(lse - posn_g)
    return np.array(loss, dtype=np.float32)


# revision 3
# speedup vs baseline: 1.2099x; 1.2099x over previous
"""NT-Xent loss on 8 Trainium2 cores (v5: dual-engine fused exp).

Structure (per core (v,s), cyclic 3-block symmetry as v4): slab s's
1024 rows against column blocks [s | s+2 | s+1] (3072 cols).  The v4
bottleneck was ScalarE doing ALL 3.1M exps (25us) while DVE spent 23us
on reductions.  v5 splits the exp work across both engines, each with a
FUSED free-dim accumulator, and moves the +1-block column sums to the
PE:

- psB half [+2b 512 | +1 1024]: ScalarE ACT Exp -> fp8, accum_out =
  rowsum (pre-quantization fp32, verified on HW).
- psA half [own 1024 | +2a 512]: custom DVE op EXP8_SUM_ANT
  (body = sq(sq(sq((x*c0 + c1)*x + c2))) ~= lam*e^(x*ASC), fitted;
  7 ALU stages + accum) -> fp8 scratch, accum_out = rowsum.
- +1-block colsums: paired DoubleRow ones-matmuls (ones [128,2,128]
  weight, rhs = two i-tiles' fp8 exp) accumulated across the 4 i-tile
  pairs into 2 persistent PSUM banks (128-replicated rows).

Host: rowsum = (accD/lam + accA) - diag + colsum[(s-1)%4]; diag is
emulated exactly (EXP8 is bit-exact vs np.float32 emulation, verified
on HW).  Shift sigma=0 so exp values are fp8-friendly (e^s, s in
[-3.3, 3.3] off-diagonal); lse = ln(rowsum + e^pos).
"""

import numpy as np
import ml_dtypes

N = 4096
D = 256
TEMP = 0.1
NCORES = 8
RPC = 2 * N // NCORES          # 1024 rows per core
IT = RPC // 128                # 8 i-tiles of 128 rows
W = 3 * RPC                    # 3072 columns per core
HALFW = W // 2                 # 1536 cols per PSUM buffer
NCH = HALFW // 512             # 3 column chunks per half
SC = 16.0                      # fp8 prescale (power of 2, exact)
ASCALE = (1.0 / TEMP) / (SC * SC)   # 10/256, exact in fp32

# EXP8 fit: (C2S + C1S*s + C0S*s^2)^8 ~= LAM * e^s  (s = G*ASCALE),
# weighted by N(0,0.625)*e^s over s in [-3.45, 3.45]
C0S, C1S, C2S = 0.00812527624, 0.125296963, 0.999881204
LAM = 1.00007132
C0G = float(np.float32(C0S * ASCALE * ASCALE))
C1G = float(np.float32(C1S * ASCALE))
C2G = float(np.float32(C2S))

ACT2_TILES = (1, 4, 6)     # tiles whose +2 block goes to ScalarE

_CACHE = {}


def _register_exp8():
    """Register the EXP8_SUM_ANT custom DVE op (runtime equivalent of the
    documented OPS.append flow; sha computed from the lowered uops)."""
    from operator import add
    from concourse.dve_spec import Spec, Src0, C0, C1, C2, Zero, sq, lower
    from concourse.dve_uop import DveOpSpec
    import concourse.dve_ops as dom

    name = "EXP8_SUM_ANT"
    for op in dom.OPS:
        if op.name == name:
            return op

    body = sq(sq(sq((Src0 * C0 + C1) * Src0 + C2)))

    def ref(in0, in1, s0, s1, imm2):
        x = in0.astype(np.float32)
        t = ((x * np.float32(s0) + np.float32(s1)) * x + np.float32(imm2)).astype(
            np.float32
        )
        t = (t * t).astype(np.float32)
        t = (t * t).astype(np.float32)
        t = (t * t).astype(np.float32)
        return t, t.reshape(t.shape[0], -1).sum(axis=-1, keepdims=True).astype(
            np.float32
        )

    spec = Spec(body=body, accum=add, accum_init=Zero, reference=ref)
    row = dom._CUSTOM_DVE_ROW_BASE + len(dom.OPS)
    dom._SUB_OPCODE_FOR_NAME[name] = row
    shas = {}
    for ver in ("v3", "v4"):
        shas[ver] = DveOpSpec(
            name=name, opcode=row, uops=lower(spec, ver=ver), rd1_en=False
        ).sha(ver)
    op = dom.DveOp(name, spec, subdim=False, uops_sha=shas)
    dom.OPS.append(op)
    dom.CUSTOM_DVE_SPECS[name] = spec
    return op


def _exp8_host(x):
    """Bit-exact host emulation of the device EXP8 body (fp32 stages)."""
    x = np.asarray(x, dtype=np.float32)
    t = ((x * np.float32(C0G) + np.float32(C1G)) * x + np.float32(C2G)).astype(
        np.float32
    )
    t = (t * t).astype(np.float32)
    t = (t * t).astype(np.float32)
    t = (t * t).astype(np.float32)
    return t


def _build_program():
    if "nc" in _CACHE:
        return _CACHE["nc"]

    import concourse.tile as tile
    from concourse import bacc, mybir

    EXP8 = _register_exp8()

    F8 = mybir.dt.float8e4
    F32 = mybir.dt.float32

    nc = bacc.Bacc(
        "TRN2", target_bir_lowering=False, debug=False, num_devices=NCORES
    )

    # anT[h][c][p][k][col] = cols[h*1536 + c*512 + col, k*128 + p]
    # column order per core: [own 1024 | +2 1024 | +1 1024]
    anT_d = nc.dram_tensor("anT", [2, NCH, 128, 2, 512], F8, kind="ExternalInput")
    # qnT[p][k][r] = q8slab[r, k*128 + p]
    qnT_d = nc.dram_tensor("qnT", [128, 2, RPC], F8, kind="ExternalInput")
    acc_d = nc.dram_tensor("acc", [128, IT, 3], F32, kind="ExternalOutput")
    cs_d = nc.dram_tensor("cs", [1, RPC], F32, kind="ExternalOutput")

    with tile.TileContext(nc) as tc:
        with (
            tc.tile_pool(name="weights", bufs=1) as wpool,
            tc.tile_pool(name="psum", bufs=4, space="PSUM") as ppool,
        ):
            qnT = wpool.tile([128, 2, RPC], F8)
            an = [
                [wpool.tile([128, 2, 512], F8, name=f"an{h}_{c}") for c in range(NCH)]
                for h in range(2)
            ]
            # block -> an tiles: own = an[0][0..1], +2 = an[0][2], an[1][0],
            # +1 = an[1][1..2].  First-use order: own, +2, +1 per tile.
            nc.sync.dma_start(out=qnT[:, :, 0:128], in_=qnT_d[:, :, 0:128])
            nc.scalar.dma_start(out=an[0][0][:], in_=anT_d[0, 0])
            nc.gpsimd.dma_start(out=an[0][1][:], in_=anT_d[0, 1])
            nc.sync.dma_start(out=an[0][2][:], in_=anT_d[0, 2])
            nc.scalar.dma_start(out=an[1][0][:], in_=anT_d[1, 0])
            nc.gpsimd.dma_start(out=an[1][1][:], in_=anT_d[1, 1])
            nc.sync.dma_start(out=qnT[:, :, 128:RPC], in_=qnT_d[:, :, 128:RPC])
            nc.scalar.dma_start(out=an[1][2][:], in_=anT_d[1, 2])

            acc = wpool.tile([128, IT, 3], F32)
            csb = wpool.tile([128, RPC], F32)
            scrD = wpool.tile([128, 1024], F8)
            scrA = wpool.tile([128, 1024], F8)
            e1full = wpool.tile([128, IT, 1024], F8)
            w1 = wpool.tile([128, 2, 128], F8)
            nc.vector.memset(w1[:], 1.0)

            # chunk c of tile t covers columns [1024c, 1024c+1024):
            # c=0 own, c=1 +2, c=2 +1; each is 2 DoubleRow MMs
            AN = [an[0] + an[1]][0]  # flat list of 6 [128,2,512] tiles

            cs = None
            first_ps = None
            for t in range(IT):
                lhsT = qnT[:, :, t * 128:(t + 1) * 128]
                for c in range(3):
                    ps = ppool.tile([128, 1024], F32, tag="ps")
                    if t == 0 and c == 0:
                        # garbage DoubleRow MMs warm the PE during input DMA;
                        # the real c-MMs (start=True) overwrite
                        with tc.high_priority():
                            for _ in range(2):
                                nc.tensor.matmul(
                                    ps[:, 0:128],
                                    w1[:],
                                    w1[:],
                                    start=True,
                                    stop=True,
                                    perf_mode=mybir.MatmulPerfMode.DoubleRow,
                                    skip_group_check=True,
                                )
                    for k in range(2):
                        nc.tensor.matmul(
                            ps[:, k * 512:(k + 1) * 512],
                            lhsT,
                            AN[2 * c + k][:],
                            start=True,
                            stop=True,
                            perf_mode=mybir.MatmulPerfMode.DoubleRow,
                            skip_group_check=True,
                        )
                    if c == 2:
                        # +1 block -> ScalarE exp (fused rowsum), kept for
                        # the deferred colsum ones-MMs
                        nc.scalar.activation(
                            e1full[:, t, :],
                            ps[:],
                            mybir.ActivationFunctionType.Exp,
                            bias=0.0,
                            scale=float(ASCALE),
                            accum_out=acc[:, t, 0:1],
                        )
                    elif c == 0 or t not in ACT2_TILES:
                        # own block (and most +2 blocks) -> DVE EXP8
                        nc.vector._custom_dve(
                            EXP8,
                            out=scrD[:],
                            in0=ps[:],
                            s0=C0G,
                            s1=C1G,
                            imm2=C2G,
                            accum_out=acc[:, t, 1 if c == 0 else 2:2 if c == 0 else 3],
                        )
                    else:
                        # +2 block on the ACT2 tiles -> ScalarE (balance)
                        nc.scalar.activation(
                            scrA[:],
                            ps[:],
                            mybir.ActivationFunctionType.Exp,
                            bias=0.0,
                            scale=float(ASCALE),
                            accum_out=acc[:, t, 2:3],
                        )
                    if t == IT - 1 and c == 0:
                        # colsum accumulator: allocated late in the rotation
                        # (only 2 allocations follow) so its slot is never
                        # reused while it must persist
                        cs = ppool.tile([128, RPC], F32, tag="ps")

            # paired DoubleRow ones-MMs: +1-block colsums, pairs (2u, 2u+1)
            assert cs is not None
            for u in range(IT // 2):
                for half in range(2):
                    nc.tensor.matmul(
                        cs[:, half * 512:(half + 1) * 512],
                        w1[:],
                        e1full[:, 2 * u:2 * u + 2, half * 512:(half + 1) * 512],
                        start=(u == 0),
                        stop=(u == IT // 2 - 1),
                        perf_mode=mybir.MatmulPerfMode.DoubleRow,
                        skip_group_check=True,
                    )

            # evacuate colsums (PSUM can't DMA): split across both engines
            nc.scalar.activation(
                csb[:, 0:512],
                cs[:, 0:512],
                mybir.ActivationFunctionType.Copy,
            )
            nc.vector.tensor_copy(csb[:, 512:RPC], cs[:, 512:RPC])

            nc.sync.dma_start(out=acc_d[:], in_=acc[:])
            nc.gpsimd.dma_start(out=cs_d[:], in_=csb[0:1, :])

    nc.compile()
    _CACHE["nc"] = nc
    return nc


def _prep_inputs(z_i, z_j):
    f8 = ml_dtypes.float8_e4m3
    zin = z_i / np.sqrt(np.sum(z_i * z_i, axis=1, keepdims=True))
    zjn = z_j / np.sqrt(np.sum(z_j * z_j, axis=1, keepdims=True))
    posn = np.sum(zin * zjn, axis=1, dtype=np.float64) / TEMP      # [4096]

    q8 = [(SC * zjn).astype(f8), (SC * zin).astype(f8)]
    # exact squared norms of the quantized rows: the device Gram diagonal
    dsq = [np.sum(b.astype(np.float64) ** 2, axis=1) for b in q8]

    in_maps = []
    for c in range(NCORES):
        v, s = divmod(c, NCORES // 2)
        b = q8[v]
        brot = np.roll(b, -s * RPC, axis=0)
        # column order: [own | +2 | +1]; +1 sits in psB at local cols
        # 512:1536 so the ones-MMs read e1[:, :, 512:1536]
        cols = np.concatenate(
            [brot[0:RPC], brot[2 * RPC:3 * RPC], brot[RPC:2 * RPC]], axis=0
        )                                               # [3072, 256]
        anT = np.ascontiguousarray(
            cols.T.reshape(2, 128, 2, NCH, 512).transpose(2, 3, 1, 0, 4)
        )
        slab = b[s * RPC:(s + 1) * RPC]
        qnT = np.ascontiguousarray(slab.T.reshape(2, 128, RPC).transpose(1, 0, 2))
        in_maps.append({"anT": anT, "qnT": qnT})
    return in_maps, posn, dsq


def kernel(z_i, z_j):
    z_i = np.asarray(z_i, dtype=np.float32)
    z_j = np.asarray(z_j, dtype=np.float32)

    from concourse.bass_utils import run_bass_kernel_spmd

    nc = _build_program()
    in_maps, posn, dsq = _prep_inputs(z_i, z_j)

    res = run_bass_kernel_spmd(nc, in_maps, list(range(NCORES)))
    _CACHE["last_results"] = res

    nv = NCORES // 2
    rowsum = np.empty(2 * N, dtype=np.float64)
    colsum = np.empty((2, nv, RPC), dtype=np.float64)
    for c in range(NCORES):
        v, s = divmod(c, nv)
        a = res.results[c]["acc"].astype(np.float64)   # [128, IT, 3]
        # acc[:,t,0] = +1 (ACT), acc[:,t,1] = own (EXP8), acc[:,t,2] = +2
        lam2 = np.array([1.0 if t in ACT2_TILES else LAM for t in range(IT)])
        rs = a[:, :, 0] + a[:, :, 1] / LAM + a[:, :, 2] / lam2[None, :]
        rowsum[c * RPC:(c + 1) * RPC] = rs.T.reshape(-1)
        colsum[v, s] = res.results[c]["cs"].astype(np.float64)[0]
    for v in range(2):
        for s in range(nv):
            # slab s's missing (s, s+3) block rowsums = colsums of the
            # +1 block computed by core (v, s-1)
            g0 = v * N + s * RPC
            rowsum[g0:g0 + RPC] += colsum[v, (s - 1) % nv]

    # exact diagonal removal: the diagonal sits in the own block (psA ->
    # EXP8); emulate the device computation bit-exactly
    dsq_g = np.concatenate(dsq).astype(np.float32)     # [8192] |q8 row|^2
    rowsum -= _exp8_host(dsq_g).astype(np.float64) / LAM

    posn_g = np.concatenate([posn, posn])
    epos_g = np.exp(posn_g)

    lse = np.log(rowsum + epos_g)
    loss = np.mean(lse - posn_g)
    return np.array(loss, dtype=np.float32)
